# revision 27
# baseline (speedup 1.0000x reference)
"""Causal attention kernel for trn2, sharded over 8 NeuronCores.

Problem (B=4, S=2048, E=2048, H=16, D=128), fp32:
    qkv = x @ w_qkv; q,k,v = split(qkv)
    q,k,v reshaped (B,S,E)->(B,H,S,D) as a RAW view (no transpose), i.e.
    per (b,h): Q_h = rows [h*128,(h+1)*128) of q[b] reinterpreted [S,D].
    o = softmax(QK^T/sqrt(D) + causal(+1/-10000)) @ V, inverse raw view,
    out = o @ w_out.

Because the raw view maps head h to a contiguous block of 128 sequence
rows, the whole computation splits into B*H = 64 independent tasks, each
touching only x[b, h*128:(h+1)*128, :] and producing
out[b, h*128:(h+1)*128, :].  Core c gets 8 tasks = rows
[c*1024,(c+1)*1024) of x.reshape(B*S, E).  No collectives.

v3: bf16 matmuls (FWL weight loads), host-side layout pre-packing so
every DMA is contiguous, single task group (weights loaded once),
causal-shrunk S/PV matmuls, x^T materialized on host, j-major Q^T/K^T
SBUF layout (contiguous PSUM->SBUF casts; the matmul reads strided),
softmax denominator accumulated on DVE+GpSimd instead of the PE,
fast approximate reciprocal.
"""

import numpy as np

B, S, E = 4, 2048, 2048
H, D, P = 16, 128, 128
NCORES = 8
NT = 8                      # tasks per core (128 rows each)
ROWS = NT * P               # 1024 rows per core
SCALE = float(1.0 / np.sqrt(D))
NEG = -1.0e9  # pre-scale additive mask; exp underflows to exactly 0.0

_NC_CACHE = {}


def build_nc(den_eng="dve16", shrink=True, jmajor=False, iters=1):
    import concourse.bass as bass
    import concourse.mybir as mybir
    import concourse.tile as tile
    from concourse import bacc

    f32 = mybir.dt.float32
    f32r = mybir.dt.float32r
    bf16 = mybir.dt.bfloat16
    AF = mybir.ActivationFunctionType
    ALU = mybir.AluOpType

    nc = bacc.Bacc("TRN2", target_bir_lowering=False, debug=False,
                   num_devices=NCORES)
    # xt: host-pretransposed x^T, layout [p=kk, kc, ti, m]
    xt = nc.dram_tensor("xt", [P, 16 * NT * P], bf16, kind="ExternalInput")
    # wqkv host layout [p, cbp(24), ko(16), 256]
    wqkv = nc.dram_tensor("wqkv", [P, 24 * 16 * 256], bf16,
                          kind="ExternalInput")
    # wout host layout [p, nch(4), co(16), 512]
    wout = nc.dram_tensor("wout", [P, 4 * 16 * 512], bf16,
                          kind="ExternalInput")
    out = nc.dram_tensor("out", [ROWS, E], f32, kind="ExternalOutput")

    xt_v = xt.ap().rearrange("p (kc t m) -> p kc (t m)", kc=16, t=NT)
    wq_v = wqkv.ap().rearrange("p (cbp ko c) -> p cbp ko c", cbp=24, ko=16)
    wo_v = wout.ap().rearrange("p (nch co n) -> p nch co n", nch=4, co=16)

    with tile.TileContext(nc) as tc:
        with (
            tc.tile_pool(name="const", bufs=1) as cpool,
            tc.tile_pool(name="persist", bufs=1) as ppool,
            tc.tile_pool(name="ot", bufs=NT) as otpool,
            tc.tile_pool(name="vn", bufs=2) as vnpool,
            tc.tile_pool(name="psA", bufs=2, space="PSUM") as psA,
            tc.tile_pool(name="psB", bufs=4 if den_eng != "pe" else 2,
                         space="PSUM") as psB,
        ):
            # maskT[kk, qq] = 0 where qq >= kk else NEG (transposed
            # orientation: partition = k, free = q).
            maskT = cpool.tile([P, P], f32, tag="maskT")
            nc.gpsimd.memset(maskT[:], 0.0)
            nc.gpsimd.affine_select(
                out=maskT[:], in_=maskT[:],
                compare_op=ALU.is_ge, fill=NEG,
                base=0, channel_multiplier=-1, pattern=[[1, P]],
            )
            # all-ones stationary for partition-sum (softmax denominator)
            ones = cpool.tile([P, P], bf16, tag="ones")
            nc.gpsimd.memset(ones[:], 1.0)
            onesf = cpool.tile([P, P], f32, tag="onesf")
            nc.gpsimd.memset(onesf[:], 1.0)
            onesr = cpool.tile([P, P], f32r, tag="onesr")
            nc.vector.tensor_copy(onesr[:], onesf[:])

            for _ in range(iters):
                # Q^T/K^T per task: j-major [d, ti, j, i] (q = i*16 + j)
                # or interleaved [d, ti, (i j)] when jmajor=False.
                qt_all = ppool.tile([P, NT, S], bf16, tag="qt")
                kt_all = ppool.tile([P, NT, S], bf16, tag="kt")
                ots = []
                with (
                    tc.tile_pool(name="vtp", bufs=1) as vtp,
                    tc.tile_pool(name="attw", bufs=4) as awpool,
                    tc.tile_pool(name="attd", bufs=2) as adpool,
                ):
                  # V^T always interleaved [d, ti, (i j)]
                  vt_all = vtp.tile([P, NT, S], bf16, tag="vt")
                  with tc.tile_pool(name="qkv", bufs=1) as qpool:
                    # x^T resident: at8[kk, kc, ti*128+m]; issued on the
                    # scalar queue so it runs parallel to sync's wq loads.
                    at8 = qpool.tile([P, 16, NT * P], bf16, tag="at8")
                    for kcg in range(8):
                        nc.scalar.dma_start(
                            at8[:, kcg * 2:(kcg + 1) * 2, :],
                            xt_v[:, kcg * 2:(kcg + 1) * 2, :])

                    dsts = {0: qt_all, 1: kt_all, 2: vt_all}

                    # ------------- QKV phase -------------
                    # col-block order: V (32..47), K (16..31), Q (0..15)
                    # so V finishes first and vnat transposes overlap K/Q
                    # matmuls.
                    with tc.tile_pool(name="wqst", bufs=2) as wst:
                        for cbp in [20, 21, 22, 23, 16, 17, 18, 19,
                                    8, 9, 10, 11, 12, 13, 14, 15,
                                    0, 1, 2, 3, 4, 5, 6, 7]:
                            wq = wst.tile([P, 16, 256], bf16, tag="wq")
                            nc.sync.dma_start(wq[:], wq_v[:, cbp, :, :])
                            for ci in range(2):
                                cb = cbp * 2 + ci
                                j = cb % 16
                                dst = dsts[cb // 16]
                                ps2 = psA.tile([P, 1024], f32,
                                               tag="mm1024")
                                # kc-outer so consecutive matmuls share
                                # the stationary operand
                                for kc in range(16):
                                    for hf in range(2):
                                        nc.tensor.matmul(
                                            ps2[:, hf * 512:
                                                (hf + 1) * 512],
                                            wq[:, kc, ci * P:(ci + 1) * P],
                                            at8[:, kc,
                                                hf * 512:(hf + 1) * 512],
                                            start=(kc == 0),
                                            stop=(kc == 15))
                                dv = dst.rearrange(
                                    "d t (i j) -> d t i j", j=16)[
                                    :, :, :, j]
                                src = ps2[:].rearrange(
                                    "d (t i) -> d t i", t=8)
                                # alternate engines: the strided
                                # interleave write is the QKV-phase
                                # bottleneck, split it DVE/ACT
                                if cb % 2 == 0:
                                    nc.vector.tensor_copy(dv, src)
                                else:
                                    nc.scalar.copy(dv, src)

                  with tc.tile_pool(name="oproj", bufs=2) as opool:
                    # -------------- attention (per task) --------------
                    for ti in range(NT):
                        # V natural [k, kt, d] for this task
                        vnat = vnpool.tile([P, 16, P], bf16, tag="vnat",
                                           name=f"vn{ti}")
                        nc.sync.dma_start_transpose(
                            vnat[:], vt_all[:, ti, :])
                        ot = otpool.tile([P, 16, P], bf16, tag="ot",
                                         name=f"ot{ti}")
                        ots.append(ot)
                        if jmajor:
                            qt_t = qt_all[:, ti, :].rearrange(
                                "d (j i) -> d i j", i=P)
                            kt_t = kt_all[:, ti, :].rearrange(
                                "d (j i) -> d i j", i=P)
                        else:
                            qt_t = qt_all[:, ti, :]
                            kt_t = kt_all[:, ti, :]

                        def s_lhsT(kt):
                            if jmajor:
                                return kt_t[:, 8 * kt:8 * (kt + 1), :]
                            return kt_t[:, kt * P:(kt + 1) * P]

                        def s_rhs(qc, w0):
                            if jmajor:
                                return qt_t[:, 32 * qc + w0 // 16:
                                            32 * (qc + 1), :]
                            return qt_t[:, qc * 512 + w0:(qc + 1) * 512]

                        for qc in range(4):
                            ot_ps = psB.tile([P, 512], f32, tag="otacc")
                            if den_eng == "pe":
                                den_ps = psB.tile([P, 512], f32,
                                                  tag="denacc")
                            else:
                                # bf16 accumulator on DVE (2x 16-bit rate)
                                acc = adpool.tile([P, 512], bf16,
                                                  tag="acc16")
                            nkt = qc * 4 + 4
                            for ktp in range(nkt // 2):
                                # pair of k-tiles -> one [P,1024] PSUM
                                # tile, ONE exp op (amortizes ScalarE's
                                # ~352-cycle fixed cost per op)
                                s2 = psA.tile([P, 1024], f32,
                                              tag="mm1024")
                                pt2 = awpool.tile([P, 1024], bf16,
                                                  tag="pt")
                                w0s = []
                                for idx in range(2):
                                    kt = 2 * ktp + idx
                                    r = kt - qc * 4
                                    w0 = P * r if (shrink and r > 0) else 0
                                    w0s.append(w0)
                                    off = idx * 512
                                    nc.tensor.matmul(
                                        s2[:, off + w0:off + 512],
                                        s_lhsT(kt), s_rhs(qc, w0),
                                        start=True, stop=True)
                                    if r >= 0:
                                        rr = off + P * r
                                        nc.vector.tensor_tensor(
                                            s2[:, rr:rr + P],
                                            s2[:, rr:rr + P],
                                            maskT[:], ALU.add)
                                e0 = w0s[0]
                                nc.scalar.activation(
                                    pt2[:, e0:1024], s2[:, e0:1024],
                                    AF.Exp, bias=1.0, scale=SCALE)
                                for idx in range(2):
                                    kt = 2 * ktp + idx
                                    w0 = w0s[idx]
                                    off = idx * 512
                                    nc.tensor.matmul(
                                        ot_ps[:, w0:512], vnat[:, kt, :],
                                        pt2[:, off + w0:off + 512],
                                        start=(kt == 0),
                                        stop=(kt == nkt - 1),
                                        skip_group_check=(w0 > 0))
                                    if den_eng == "pe":
                                        nc.tensor.matmul(
                                            den_ps[:, w0:512], ones[:],
                                            pt2[:, off + w0:off + 512],
                                            start=(kt == 0),
                                            stop=(kt == nkt - 1),
                                            skip_group_check=(w0 > 0))
                                    elif kt == 0:
                                        nc.vector.tensor_copy(
                                            acc[:], pt2[:, 0:512])
                                    else:
                                        nc.vector.tensor_tensor(
                                            acc[:, w0:512],
                                            acc[:, w0:512],
                                            pt2[:, off + w0:off + 512],
                                            ALU.add)
                            if den_eng == "pe":
                                den_fin = den_ps
                            else:
                                dfw = psA.tile([P, 1024], f32,
                                               tag="mm1024")
                                den_fin = dfw[:, 0:512]
                                nc.tensor.matmul(
                                    den_fin, ones[:], acc[:],
                                    start=True, stop=True)
                            rec = adpool.tile([P, 512], f32, tag="rec")
                            nc.vector.reciprocal_approx_fast(
                                rec[:], den_fin[:] if den_eng == "pe"
                                else den_fin)
                            nc.vector.tensor_tensor(
                                ot[:, qc * 4:(qc + 1) * 4, :].rearrange(
                                    "p s d -> p (s d)"),
                                ot_ps[:], rec[:], ALU.mult)

                    # ---------------- output projection ----------------
                    for nch in range(4):
                        wo = opool.tile([P, 16, 512], bf16, tag="wo")
                        nc.sync.dma_start(wo[:], wo_v[:, nch, :, :])
                        for ti in range(NT):
                            lt = ots[ti].rearrange(
                                "d qt (i j) -> d qt i j", j=16)
                            psw = psA.tile([P, 1024], f32, tag="mm1024")
                            ps = psw[:, 0:512]
                            for cc in range(16):
                                nc.tensor.matmul(
                                    ps, lt[:, :, :, cc],
                                    wo[:, cc, :],
                                    start=(cc == 0), stop=(cc == 15))
                            osb = opool.tile([P, 512], f32, tag="osb")
                            nc.scalar.copy(osb[:], ps)
                            nc.scalar.dma_start(
                                out.ap()[ti * P:(ti + 1) * P,
                                         nch * 512:(nch + 1) * 512],
                                osb[:])
    nc.compile()
    return nc


def _env_opts():
    import os
    opts = {}
    for k in ("shrink", "jmajor"):
        v = os.environ.get("BK_" + k.upper())
        if v is not None:
            opts[k] = v not in ("0", "false", "False")
    v = os.environ.get("BK_DEN_ENG")
    if v:
        opts["den_eng"] = v
    return opts


def get_nc(**kw):
    opts = _env_opts()
    opts.update(kw)
    key = tuple(sorted(opts.items()))
    if key not in _NC_CACHE:
        _NC_CACHE[key] = build_nc(**opts)
    return _NC_CACHE[key]


def _prep_inputs(x, w_qkv, w_out):
    """Host-side dtype conversion + layout pre-packing (bf16)."""
    import ml_dtypes

    bf = ml_dtypes.bfloat16
    x = np.asarray(x, dtype=np.float32).reshape(NCORES, NT, P, 16, P)
    # xt[core][p, kc, ti, m] = x[core, ti, m, kc, p]
    xt = np.ascontiguousarray(x.transpose(0, 4, 3, 1, 2)).astype(bf)
    xt = xt.reshape(NCORES, P, 16 * NT * P)
    w = np.asarray(w_qkv, dtype=np.float32).reshape(16, P, 24, 256)
    wq = np.ascontiguousarray(w.transpose(1, 2, 0, 3)).astype(bf)
    wq = wq.reshape(P, 24 * 16 * 256)
    wo = np.asarray(w_out, dtype=np.float32).reshape(16, P, 4, 512)
    wo = np.ascontiguousarray(wo.transpose(1, 2, 0, 3)).astype(bf)
    wo = wo.reshape(P, 4 * 16 * 512)
    return xt, wq, wo


def kernel(x, w_qkv, w_out):
    from concourse.bass_utils import run_bass_kernel_spmd

    xt, wq, wo = _prep_inputs(x, w_qkv, w_out)
    nc = get_nc()
    in_maps = [
        {"xt": np.ascontiguousarray(xt[c]), "wqkv": wq, "wout": wo}
        for c in range(NCORES)
    ]
    res = run_bass_kernel_spmd(nc, in_maps, core_ids=list(range(NCORES)))
    outs = [res.results[c]["out"] for c in range(NCORES)]
    return np.concatenate(outs, axis=0).reshape(B, S, E).astype(np.float32)
